# revision 1
# baseline (speedup 1.0000x reference)
"""ConvKAN Trainium2 kernel.

Decomposition (validated to 3e-7 vs reference):
  out[(b, cin, kh, kw, q), oc] =
      sum_{func, dh, dw} Wf[oc, func, dh*48+dw] * F_func(x_pad[b, cin, 12q+kh+dh, kw+dw])
  where F_0 = silu and F_{1+g}(v) = (r2^3 - 4*r1^3)/6 with
  t = |2.5 v + 3.5 - g|, r2 = max(2-t, 0), r1 = max(1-t, 0)  (uniform cubic B-spline).

Sharding: input channels cin split 8 ways (8 per core); because the reference's
"faithful" view(-1, in_dim) maps row blocks to output channels, core k produces
output channels [16k, 16k+16) for all batches.

Device layout: features packed on partitions as (func-slot, w_pad) with 64-wide
slots; contraction per matmul = (2 funcs x 64 w); accumulate over dh (12) and
5 chunk tensors into PSUM per output group (kh, kw); kw is folded into
host-prepared zero-padded weights; kh, dh, q ride the free-dim h access pattern.
"""

from contextlib import ExitStack

import numpy as np

import concourse.bass as bass
import concourse.bacc as bacc
import concourse.tile as tile
from concourse import mybir
from concourse.alu_op_type import AluOpType
from concourse.bass_utils import run_bass_kernel_spmd

AF = mybir.ActivationFunctionType
DT = mybir.dt

B, C, H, W = 16, 64, 48, 48
OUT_C = 128
NCORES = 8
CLOC = C // NCORES          # 8 input channels per core
BC = B * CLOC               # 128 (b, c) pairs per core
HP = 50                     # padded height
WSLOT = 64                  # padded width slot on partitions
FREE = HP * BC              # 6400
A3 = 4.0 ** (1.0 / 3.0)
RUN_KWARGS = {}
LAST_EXEC_NS = None
N_IDX = 3 * 12 * 5          # kw * dh * chunk lhsT tiles
# chunk -> (funcA, funcB) with func 0 = silu, 1+g = basis g; None = zero pad
CHUNK_FUNCS = [(0, None), (1, 2), (3, 4), (5, 6), (7, 8)]


def build_nc(fch: int = 8) -> bass.Bass:
    nc = bacc.Bacc(None, target_bir_lowering=False, debug=True)
    xT = nc.declare_dram_parameter("xT", [128, FREE], DT.float32, isOutput=False)
    wq = nc.declare_dram_parameter("wq", [128, N_IDX * 128], DT.float16, isOutput=False)
    bias = nc.declare_dram_parameter("bias", [128, 8], DT.float32, isOutput=False)
    out = nc.declare_dram_parameter("out", [9, 128, 512], DT.float32, isOutput=True)

    with ExitStack() as ctx:
        tc = ctx.enter_context(tile.TileContext(nc))
        wpool = ctx.enter_context(tc.tile_pool(name="w", bufs=1))
        fpool = ctx.enter_context(tc.tile_pool(name="f", bufs=2))
        psum_pool = ctx.enter_context(tc.tile_pool(name="ps", bufs=8, space="PSUM"))
        opool = ctx.enter_context(tc.tile_pool(name="o", bufs=4))

        wq_sb = wpool.tile([128, N_IDX * 128], DT.float16)
        for ch in range(5):
            wsl = slice(ch * 36 * 128, (ch + 1) * 36 * 128)
            nc.sync.dma_start(wq_sb[:, wsl], wq[:, wsl])
        x_sb = wpool.tile([128, FREE], DT.float32)
        bias_sb = wpool.tile([128, 8], DT.float32)
        nc.gpsimd.dma_start(bias_sb[:], bias[:])

        ts_t = [wpool.tile([128, FREE], DT.float16, name=f"ts{k}", tag=f"ts{k}")
                for k in range(5)]

        # chunk 0: silu = x * sigmoid(x) (partitions >=64 get dup - zero weights)
        fw = FREE // fch
        for f in range(fch):
            sl = slice(f * fw, (f + 1) * fw)
            nc.sync.dma_start(x_sb[:, sl], xT[:, sl])
            sig = fpool.tile([128, fw], DT.float32, tag="sig")
            nc.scalar.activation(sig[:], x_sb[:, sl], AF.Sigmoid)
            nc.vector.tensor_tensor(ts_t[0][:, sl], x_sb[:, sl], sig[:],
                                    op=AluOpType.mult)

        # basis chunks: TS value = 4 r1^3 - r2^3 (spline weights negated)
        for k in range(1, 5):
            for f in range(fch):
                sl = slice(f * fw, (f + 1) * fw)
                t = fpool.tile([128, fw], DT.float16, tag="t")
                nc.scalar.activation(t[:], x_sb[:, sl], AF.Abs,
                                     bias=bias_sb[:, k - 1:k], scale=2.5)
                nr2 = fpool.tile([128, fw], DT.float16, tag="nr2")  # -r2 = min(t-2, 0)
                nc.vector.tensor_scalar(nr2[:], t[:], 2.0, 0.0,
                                        op0=AluOpType.subtract, op1=AluOpType.min)
                nr1 = fpool.tile([128, fw], DT.float16, tag="nr1")  # -r1 = min(t-1, 0)
                nc.vector.tensor_scalar(nr1[:], t[:], 1.0, 0.0,
                                        op0=AluOpType.subtract, op1=AluOpType.min)
                s2 = fpool.tile([128, fw], DT.float16, tag="s2")    # r2^2
                nc.vector.tensor_tensor(s2[:], nr2[:], nr2[:], op=AluOpType.mult)
                s1f = fpool.tile([128, fw], DT.float16, tag="s1f")  # 4 r1^2
                nc.scalar.activation(s1f[:], nr1[:], AF.Square, scale=2.0)
                c2n = fpool.tile([128, fw], DT.float16, tag="c2n")  # -r2^3
                nc.vector.tensor_tensor(c2n[:], s2[:], nr2[:], op=AluOpType.mult)
                cn1 = fpool.tile([128, fw], DT.float16, tag="cn1")  # -4 r1^3
                nc.vector.tensor_tensor(cn1[:], s1f[:], nr1[:], op=AluOpType.mult)
                # 4 r1^3 - r2^3 = c2n - cn1
                nc.vector.tensor_tensor(ts_t[k][:, sl], c2n[:], cn1[:],
                                        op=AluOpType.subtract)

        # matmuls: 9 output groups (kh, kw); accumulate 12 dh x 5 chunks.
        # Two PSUM waves; within a wave, MMs ordered chunk-major so PE can
        # start as soon as the first chunk tensor is complete.
        groups = [(kh, kw) for kh in range(3) for kw in range(3)]
        waves = [groups[:4], groups[4:8], groups[8:]]
        ps_tiles = {}

        def emit_wave(wave):
            for ch in range(5):
                for (kh, kw) in wave:
                    ps3 = ps_tiles[(kh, kw)][:].rearrange("p (q b) -> p q b", b=BC)
                    for dh in range(12):
                        h0 = kh + dh
                        idx = (ch * 3 + kw) * 12 + dh
                        lhsT = wq_sb[:, idx * 128:(idx + 1) * 128]
                        rhs = ts_t[ch][:].rearrange("p (h b) -> p h b", b=BC)[
                            :, h0:h0 + 37:12, :]
                        nc.tensor.matmul(ps3, lhsT, rhs,
                                         start=(ch == 0 and dh == 0),
                                         stop=(ch == 4 and dh == 11))

        def drain_wave(wave):
            for (kh, kw) in wave:
                ob = opool.tile([128, 512], DT.float32)
                nc.scalar.copy(ob[:], ps_tiles[(kh, kw)][:])
                nc.gpsimd.dma_start(out[kh * 3 + kw], ob[:])

        for g in waves[0] + waves[1]:
            ps_tiles[g] = psum_pool.tile([128, 512], DT.float32,
                                         name=f"ps_{g[0]}{g[1]}", tag="ps")
        # interleave the two leading waves chunk-by-chunk
        for ch in range(5):
            for wave in (waves[0], waves[1]):
                for (kh, kw) in wave:
                    ps3 = ps_tiles[(kh, kw)][:].rearrange("p (q b) -> p q b", b=BC)
                    for dh in range(12):
                        h0 = kh + dh
                        idx = (ch * 3 + kw) * 12 + dh
                        lhsT = wq_sb[:, idx * 128:(idx + 1) * 128]
                        rhs = ts_t[ch][:].rearrange("p (h b) -> p h b", b=BC)[
                            :, h0:h0 + 37:12, :]
                        nc.tensor.matmul(ps3, lhsT, rhs,
                                         start=(ch == 0 and dh == 0),
                                         stop=(ch == 4 and dh == 11))
        drain_wave(waves[0])
        drain_wave(waves[1])
        for g in waves[2]:
            ps_tiles[g] = psum_pool.tile([128, 512], DT.float32,
                                         name=f"ps_{g[0]}{g[1]}", tag="ps")
        emit_wave(waves[2])
        drain_wave(waves[2])
    nc.compile()
    return nc


def _prep_weights(base_weight, spline_weight, spline_scaler):
    # Wf[oc, func, jj]: func 0 = silu weights, 1+g = scaled spline / 6
    wf = np.empty((OUT_C, 9, 576), dtype=np.float64)
    wf[:, 0, :] = base_weight
    wf[:, 1:, :] = np.moveaxis(
        spline_weight.astype(np.float64)
        * spline_scaler.astype(np.float64)[..., None] / -6.0, -1, 1)
    w4 = wf.reshape(OUT_C, 9, 12, 48)
    # wq[p, idx=(kw,dh,chunk), oc]
    wq = np.zeros((128, 5, 3, 12, OUT_C), dtype=np.float64)
    for kw in range(3):
        for ch, (fa, fb) in enumerate(CHUNK_FUNCS):
            for s, func in enumerate((fa, fb)):
                if func is None:
                    continue
                # partition p = s*64 + wp; weight = w4[oc, func, dh, wp-kw], 0<=wp-kw<48
                for wp in range(kw, kw + 48):
                    wq[s * 64 + wp, ch, kw, :, :] = w4[:, func, :, wp - kw].T
    wq = wq.reshape(128, 5 * 3 * 12 * 128).astype(np.float16)
    bias = np.zeros((128, 8), dtype=np.float32)
    bias[:, 4] = -A3
    for k in range(1, 5):
        ga, gb = CHUNK_FUNCS[k]
        bias[:64, k - 1] = 3.5 - (ga - 1)
        bias[64:, k - 1] = 3.5 - (gb - 1)
    return wq, bias


def _prep_x(x_slice):
    # x_slice: (B, CLOC, 48, 48) float32 -> (128, FREE) with dup halves
    xp = np.zeros((WSLOT, HP, BC), dtype=np.float32)
    xp[1:49, 1:49, :] = np.ascontiguousarray(x_slice.transpose(3, 2, 0, 1)).reshape(48, 48, BC)
    flat = xp.reshape(WSLOT, FREE)
    return np.concatenate([flat, flat], axis=0)


def kernel(x, base_weight, spline_weight, spline_scaler):
    x = np.asarray(x, dtype=np.float32)
    wq, bias = _prep_weights(np.asarray(base_weight), np.asarray(spline_weight),
                             np.asarray(spline_scaler))
    nc = build_nc()
    in_maps = []
    for k in range(NCORES):
        xs = x[:, k * CLOC:(k + 1) * CLOC]
        in_maps.append({"xT": _prep_x(xs), "wq": wq, "bias": bias})
    res = run_bass_kernel_spmd(nc, in_maps, list(range(NCORES)), **RUN_KWARGS)
    global LAST_EXEC_NS
    LAST_EXEC_NS = res.exec_time_ns
    outs = [np.asarray(r["out"]) for r in res.results]

    full = np.empty((B, 2304, OUT_C), dtype=np.float32)
    for k in range(NCORES):
        dev = outs[k].reshape(3, 3, OUT_C, 4, B, CLOC)
        rows = dev.transpose(4, 5, 0, 1, 3, 2).reshape(B, 288, OUT_C)
        full[:, 288 * k:288 * (k + 1), :] = rows
    return full.reshape(B, 128, 2304).reshape(B, 128, 48, 48)



# revision 4
# speedup vs baseline: 1.0690x; 1.0690x over previous
"""ConvKAN Trainium2 kernel (v2: 4-pass func packing).

Decomposition (validated vs reference):
  out[(b, cin, kh, kw, q), oc] =
      sum_{func, jh, jw} Wf[oc, func, jh*48+jw] * F_func(x_pad[b, cin, 12q+jh+kh, jw+kw])
  where F_0 = silu and F_{1+g}(v) = (r2^3 - 4*r1^3)/(-6) spline cubes with
  t = |2.5 v + 3.5 - g|, r2 = max(2-t, 0), r1 = max(1-t, 0).

Sharding: input channels cin split 8 ways (8 per core); the reference's
"faithful" view(-1, in_dim) maps row blocks to output channels, so core k
produces output rows [288k, 288k+288) of (B, 2304, OUT_C).

Device layout (per core): 4 activation passes; pass c partitions:
  p in [28,78):  spline basis g = 2c   at w_pad = p-28  (50-wide padded w)
  p in [78,128): spline basis g = 2c+1 at w_pad = p-78
  pass 0, p in [0,28): silu at w_pad = p      (w 0..27)
  pass 1, p in [0,22): silu at w_pad = 28+p   (w 28..49)
  (all other slots: finite junk x zero weights)
The spline chain runs on all 128 partitions (free-dim bound, so free); the
silu rows are overwritten afterwards by one ACT Silu op at partition base 0.
Accumulate over pass (4) x jh (12) into PSUM per output group (kh, kw); kw is
folded into host-prepared zero-padded weights; kh, jh, q ride the free-dim h
access pattern (stride 12 over 50 h rows).
"""

from contextlib import ExitStack

import numpy as np

import concourse.bass as bass
import concourse.bacc as bacc
import concourse.tile as tile
from concourse import mybir
from concourse.alu_op_type import AluOpType
from concourse.bass_utils import run_bass_kernel_spmd

AF = mybir.ActivationFunctionType
DT = mybir.dt

B, C, H, W = 16, 64, 48, 48
OUT_C = 128
NCORES = 8
CLOC = C // NCORES          # 8 input channels per core
BC = B * CLOC               # 128 (b, c) pairs per core
HP = 50                     # padded height
FREE = HP * BC              # 6400
NPASS = 4
NTILE = NPASS * 3 * 12      # wq lhsT tiles (pass, kw, jh)
FCH = 4                     # free-dim chunks per activation pass
RUN_KWARGS = {}
LAST_EXEC_NS = None
N_WARMUP = 22               # HAM warm-up dummy matmuls

# flexible engine assignment knobs (per (pass, chunk) flattened index 0..15)
SUB_ON_GPSIMD = 10          # first N subs on gpsimd, rest on vector
S1F_ON_DVE = 4              # last N s1f squares via DVE 3-op path


def build_nc(fch: int = FCH) -> bass.Bass:
    nc = bacc.Bacc(None, target_bir_lowering=False, debug=True)
    xT = nc.declare_dram_parameter("xT", [128, 2 * FREE], DT.float16, isOutput=False)
    wq = nc.declare_dram_parameter("wq", [128, NTILE * 128], DT.float16, isOutput=False)
    bias = nc.declare_dram_parameter("bias", [128, 8], DT.float32, isOutput=False)
    out = nc.declare_dram_parameter("out", [9, 128, 512], DT.float32, isOutput=True)

    fw = FREE // fch
    with ExitStack() as ctx:
        tc = ctx.enter_context(tile.TileContext(nc))
        wpool = ctx.enter_context(tc.tile_pool(name="w", bufs=1))
        fpool = ctx.enter_context(tc.tile_pool(name="f", bufs=2))
        psum_pool = ctx.enter_context(tc.tile_pool(name="ps", bufs=8, space="PSUM"))
        opool = ctx.enter_context(tc.tile_pool(name="o", bufs=4))

        x_sb = wpool.tile([128, 2 * FREE], DT.float16)
        bias_sb = wpool.tile([128, 8], DT.float32)
        nc.gpsimd.dma_start(bias_sb[:], bias[:])
        # x first: main region in f-chunks, then the pass-1 silu block
        for f in range(fch):
            sl = slice(f * fw, (f + 1) * fw)
            nc.sync.dma_start(x_sb[:, sl], xT[:, sl])
        nc.sync.dma_start(x_sb[0:22, FREE:2 * FREE], xT[0:22, FREE:2 * FREE])

        wq_sb = wpool.tile([128, NTILE * 128], DT.float16)
        for c in range(NPASS):
            wsl = slice(c * 36 * 128, (c + 1) * 36 * 128)
            nc.sync.dma_start(wq_sb[:, wsl], wq[:, wsl])

        ts_t = [wpool.tile([128, FREE], DT.float16, name=f"ts{c}", tag=f"ts{c}")
                for c in range(NPASS)]

        # HAM warm-up: dummy matmuls on x data into the group-7 psum bank
        # (cleared by group 7's first real start=True matmul).
        groups = [(kh, kw) for kh in range(3) for kw in range(3)]
        ps_tiles = {}
        for g in groups[:8]:
            ps_tiles[g] = psum_pool.tile([128, 512], DT.float32,
                                         name=f"ps_{g[0]}{g[1]}", tag="ps")
        warm = ps_tiles[groups[7]][:]
        for i in range(N_WARMUP):
            nc.tensor.matmul(warm, x_sb[:, 0:128], x_sb[:, 512:1024],
                             start=True, stop=False)

        # activation chains: per pass c, all 128 partitions (silu rows junk)
        kidx = 0
        for c in range(NPASS):
            bias_ap = bias_sb[:, c:c + 1]
            for f in range(fch):
                sl = slice(f * fw, (f + 1) * fw)
                t = fpool.tile([128, fw], DT.float16, tag="t")
                nc.scalar.activation(t[:], x_sb[:, sl], AF.Abs,
                                     bias=bias_ap, scale=2.5)
                nr2 = fpool.tile([128, fw], DT.float16, tag="nr2")  # -r2
                nc.vector.tensor_scalar(nr2[:], t[:], 2.0, 0.0,
                                        op0=AluOpType.subtract, op1=AluOpType.min)
                nr1 = fpool.tile([128, fw], DT.float16, tag="nr1")  # -r1
                nc.vector.tensor_scalar(nr1[:], t[:], 1.0, 0.0,
                                        op0=AluOpType.subtract, op1=AluOpType.min)
                s2 = fpool.tile([128, fw], DT.float16, tag="s2")    # r2^2
                nc.vector.tensor_tensor(s2[:], nr2[:], nr2[:], op=AluOpType.mult)
                c2n = fpool.tile([128, fw], DT.float16, tag="c2n")  # -r2^3
                nc.vector.tensor_tensor(c2n[:], s2[:], nr2[:], op=AluOpType.mult)
                s1f = fpool.tile([128, fw], DT.float16, tag="s1f")  # 4 r1^2
                if kidx >= NPASS * fch - S1F_ON_DVE:
                    nr14 = fpool.tile([128, fw], DT.float16, tag="nr14")
                    nc.vector.tensor_scalar(nr14[:], nr1[:], 4.0, 0.0,
                                            op0=AluOpType.mult, op1=AluOpType.add)
                    nc.vector.tensor_tensor(s1f[:], nr14[:], nr1[:],
                                            op=AluOpType.mult)
                else:
                    nc.scalar.activation(s1f[:], nr1[:], AF.Square, scale=2.0)
                cn1 = fpool.tile([128, fw], DT.float16, tag="cn1")  # -4 r1^3
                nc.vector.tensor_tensor(cn1[:], s1f[:], nr1[:], op=AluOpType.mult)
                # ts = 4 r1^3 - r2^3 = c2n - cn1 (weights hold -basis/6 scale)
                eng = nc.gpsimd if kidx < SUB_ON_GPSIMD else nc.vector
                eng.tensor_tensor(ts_t[c][:, sl], c2n[:], cn1[:],
                                  op=AluOpType.subtract)
                kidx += 1
            # silu rows overwrite the junk at partition base 0
            if c == 0:
                for f in range(fch):
                    sl = slice(f * fw, (f + 1) * fw)
                    nc.scalar.activation(ts_t[0][0:28, sl], x_sb[0:28, sl],
                                         AF.Silu)
            elif c == 1:
                for f in range(fch):
                    sl = slice(f * fw, (f + 1) * fw)
                    nc.scalar.activation(ts_t[1][0:22, sl],
                                         x_sb[0:22, FREE + f * fw:FREE + (f + 1) * fw],
                                         AF.Silu)

        # matmuls: groups 0..7 accumulate pass-major; group 8 ((2,2)) runs
        # in the bank freed by group 0's drain.
        def emit_mm(g, c, kw, jh):
            kh = g[0]
            idx = (c * 3 + kw) * 12 + jh
            lhsT = wq_sb[:, idx * 128:(idx + 1) * 128]
            h0 = kh + jh
            rhs = ts_t[c][:].rearrange("p (h b) -> p h b", b=BC)[
                :, h0:h0 + 37:12, :]
            ps3 = ps_tiles[g][:].rearrange("p (q b) -> p q b", b=BC)
            nc.tensor.matmul(ps3, lhsT, rhs,
                             start=(c == 0 and jh == 0),
                             stop=(c == NPASS - 1 and jh == 11))

        def drain(g, di):
            ob = opool.tile([128, 512], DT.float32)
            if di % 2 == 0:
                nc.scalar.copy(ob[:], ps_tiles[g][:])
            else:
                nc.vector.tensor_copy(ob[:], ps_tiles[g][:])
            nc.sync.dma_start(out[g[0] * 3 + g[1]], ob[:])

        wave = groups[:8]
        for c in range(NPASS - 1):
            for kw in range(3):
                for jh in range(12):
                    for kh in range(3):
                        if (kh, kw) in wave:
                            emit_mm((kh, kw), c, kw, jh)
        # last pass: group 0 first, drain it, then the rest, then group 8
        c = NPASS - 1
        g0 = wave[0]
        for jh in range(12):
            emit_mm(g0, c, g0[1], jh)
        drain(g0, 0)
        for gi, g in enumerate(wave[1:], 1):
            for jh in range(12):
                emit_mm(g, c, g[1], jh)
            drain(g, gi)
        g8 = groups[8]
        ps_tiles[g8] = psum_pool.tile([128, 512], DT.float32, name="ps_22",
                                      tag="ps")
        for c in range(NPASS):
            for jh in range(12):
                emit_mm(g8, c, g8[1], jh)
        drain(g8, 2)
    nc.compile()
    return nc


def _slot(c, p):
    """(func, w_pad) for pass c partition p; func 0=silu, 1+g=spline; None=dead."""
    if 28 <= p < 78:
        return 1 + 2 * c, p - 28
    if p >= 78:
        return 1 + 2 * c + 1, p - 78
    if c == 0:
        return 0, p
    if c == 1 and p < 22:
        return 0, 28 + p
    return None, None


def _prep_weights(base_weight, spline_weight, spline_scaler):
    # Wf[oc, func, jj]: func 0 = silu weights, 1+g = scaled spline / -6
    wf = np.empty((OUT_C, 9, 576), dtype=np.float64)
    wf[:, 0, :] = base_weight
    wf[:, 1:, :] = np.moveaxis(
        spline_weight.astype(np.float64)
        * spline_scaler.astype(np.float64)[..., None] / -6.0, -1, 1)
    w4 = wf.reshape(OUT_C, 9, 12, 48)
    # wq[p, idx=(pass,kw,jh), oc]
    wq = np.zeros((128, NPASS, 3, 12, OUT_C), dtype=np.float64)
    for c in range(NPASS):
        for p in range(128):
            func, wp = _slot(c, p)
            if func is None:
                continue
            for kw in range(3):
                jw = wp - kw
                if 0 <= jw < 48:
                    wq[p, c, kw, :, :] = w4[:, func, :, jw].T
    wq = wq.reshape(128, NTILE * 128).astype(np.float16)
    bias = np.zeros((128, 8), dtype=np.float32)
    for c in range(NPASS):
        bias[28:78, c] = 3.5 - 2 * c
        bias[78:128, c] = 3.5 - (2 * c + 1)
    return wq, bias


def _prep_x(x_slice):
    # x_slice: (B, CLOC, 48, 48) -> xT (128, 2*FREE) fp16
    xp = np.zeros((HP, HP, BC), dtype=np.float32)
    xp[1:49, 1:49, :] = np.ascontiguousarray(
        x_slice.transpose(3, 2, 0, 1)).reshape(48, 48, BC)
    flat = xp.reshape(HP, FREE)
    xT = np.zeros((128, 2 * FREE), dtype=np.float16)
    xT[0:28, :FREE] = flat[0:28]
    xT[28:78, :FREE] = flat
    xT[78:128, :FREE] = flat
    xT[0:22, FREE:] = flat[28:50]
    return xT


def kernel(x, base_weight, spline_weight, spline_scaler):
    x = np.asarray(x, dtype=np.float32)
    wq, bias = _prep_weights(np.asarray(base_weight), np.asarray(spline_weight),
                             np.asarray(spline_scaler))
    nc = build_nc()
    in_maps = []
    for k in range(NCORES):
        xs = x[:, k * CLOC:(k + 1) * CLOC]
        in_maps.append({"xT": _prep_x(xs), "wq": wq, "bias": bias})
    res = run_bass_kernel_spmd(nc, in_maps, list(range(NCORES)), **RUN_KWARGS)
    global LAST_EXEC_NS
    LAST_EXEC_NS = res.exec_time_ns
    outs = [np.asarray(r["out"]) for r in res.results]

    full = np.empty((B, 2304, OUT_C), dtype=np.float32)
    for k in range(NCORES):
        dev = outs[k].reshape(3, 3, OUT_C, 4, B, CLOC)
        rows = dev.transpose(4, 5, 0, 1, 3, 2).reshape(B, 288, OUT_C)
        full[:, 288 * k:288 * (k + 1), :] = rows
    return full.reshape(B, 128, 2304).reshape(B, 128, 48, 48)
